# revision 23
# baseline (speedup 1.0000x reference)
# MultiHeadAttention forward, sharded over 8 NeuronCores.
#
# Problem: N=4, S=T=2048, E=1024, H=16 heads (DH=64), fp32 in/out.
# Sharding: core = n*2 + hg  (n = batch 0..3, hg = head-group 0..1, 8 heads each).
# Each core computes q/k/v projections for its head group, attention, and a
# partial output projection; the host sums the two partials per batch and adds bp.
#
# Device layout choices (all contractions need the reduced dim on partitions):
#   - host passes query/key/value TRANSPOSED ([E, S]) and the weights
#     pre-transposed, so every matmul operand loads naturally.
#   - qT, kT computed transposed [512, S] (head-dim on partitions); v natural
#     [T, 512]; scores computed transposed [T, S] so softmax normalization can
#     be deferred: exp() is applied PSUM->SBUF on the Scalar engine (scale=1/8
#     fused), y = v_aug.T @ expT accumulates over T with a ones-column in v_aug
#     producing the softmax denominators for free in psum row 64.
#   - K=64 score matmuls are row-packed in pairs of heads (partitions 0-63 /
#     64-127) for PE subarray concurrency.
#   - emission is software-pipelined (scores of group g+1 before y of group g;
#     qk projections of pair j+1 interleaved ahead of attention of pair j) so
#     the in-order PE stream never waits on the Scalar engine's exp.

import numpy as np
import ml_dtypes

N, S, T, E, H = 4, 2048, 2048, 1024, 16
DH = E // H  # 64
HC = H // 2  # heads per core = 8
EC = HC * DH  # per-core head width = 512
SCALE = float(np.sqrt(DH))  # 8.0
P = 128
KT = E // P  # 8 contraction tiles for projections
NT = T // P  # 16 T-tiles
NC_ = 8  # cores

# matmul/IO dtype on device
_DT_NAME = "bfloat16"

_cache = {}


def _build_nc(debug_dumps=False, reps=1):
    import concourse.bass as bass
    import concourse.mybir as mybir
    import concourse.tile as tile
    from concourse import bacc

    DT = getattr(mybir.dt, _DT_NAME)
    F32 = mybir.dt.float32
    AF = mybir.ActivationFunctionType
    ALU = mybir.AluOpType

    nc = bacc.Bacc("TRN2", target_bir_lowering=False, debug=False, num_devices=NC_)

    qT_in = nc.dram_tensor("qT_in", [E, S], DT, kind="ExternalInput")
    kT_in = nc.dram_tensor("kT_in", [E, T], DT, kind="ExternalInput")
    vT_in = nc.dram_tensor("vT_in", [E, T], DT, kind="ExternalInput")
    wqT = nc.dram_tensor("wqT", [E, EC], DT, kind="ExternalInput")
    wkT = nc.dram_tensor("wkT", [E, EC], DT, kind="ExternalInput")
    wvT = nc.dram_tensor("wvT", [E, EC], DT, kind="ExternalInput")
    wpT = nc.dram_tensor("wpT", [EC, E], DT, kind="ExternalInput")
    bq_d = nc.dram_tensor("bq", [EC], F32, kind="ExternalInput")
    bk_d = nc.dram_tensor("bk", [EC], F32, kind="ExternalInput")
    bv_d = nc.dram_tensor("bv", [EC], F32, kind="ExternalInput")
    out_d = nc.dram_tensor("out", [S, E], F32, kind="ExternalOutput")
    if debug_dumps:
        dbg_qt = nc.dram_tensor("dbg_qt", [4, P, S], F32, kind="ExternalOutput")
        dbg_kt = nc.dram_tensor("dbg_kt", [4, P, T], F32, kind="ExternalOutput")
        dbg_v = nc.dram_tensor(
            "dbg_v", [NT, P, HC * (DH + 1)], F32, kind="ExternalOutput"
        )
        dbg_yt = nc.dram_tensor("dbg_yt", [4, P, S], F32, kind="ExternalOutput")

    SCH = 512  # S-chunk width (scores rhs free size)
    NCH = S // SCH  # 4 chunks
    TG = 2  # T-tiles per exp group
    NG = NT // TG  # 8 groups

    with tile.TileContext(nc) as tc:
        with (
            tc.tile_pool(name="const", bufs=1) as const,
            tc.tile_pool(name="wbuf", bufs=17) as wbuf,
            tc.tile_pool(name="inbuf", bufs=17) as inbuf,
            tc.tile_pool(name="wp", bufs=1) as wppool,
            tc.tile_pool(name="qt", bufs=1) as qtpool,
            tc.tile_pool(name="kt", bufs=1) as ktpool,
            tc.tile_pool(name="vsb", bufs=1) as vpool,
            tc.tile_pool(name="yt", bufs=1) as ytpool,
            tc.tile_pool(name="expb", bufs=4) as expool,
            tc.tile_pool(name="norm", bufs=2) as norm,
            tc.tile_pool(name="ostage", bufs=2) as ostage,
            tc.tile_pool(name="dbgp", bufs=2 if debug_dumps else 1) as dbgp,
            tc.tile_pool(name="ps_sc", bufs=3, space="PSUM") as ps_sc,
            tc.tile_pool(name="ps_y", bufs=1, space="PSUM") as ps_y,
        ):
            for _rep in range(reps):
                # ---- constants / biases ----
                bq_sb = const.tile([P, KT // 2], F32, tag="bq")  # [128, 4]
                bk_sb = const.tile([P, KT // 2], F32, tag="bk")
                bvrow = const.tile([P, EC], F32, tag="bvrow")
                bvrep = const.tile([P, EC], F32, tag="bvrep")
                nc.sync.dma_start(bq_sb[:, :], bq_d.rearrange("(m p) -> p m", p=P))
                nc.sync.dma_start(bk_sb[:, :], bk_d.rearrange("(m p) -> p m", p=P))
                nc.sync.dma_start(bvrow[0:1, :], bv_d[None, :])
                nc.gpsimd.partition_broadcast(bvrep[:, :], bvrow[0:1, :])

                # persistent activation tensors
                qt_t = [
                    qtpool.tile([P, S], DT, tag=f"qt{j}", name=f"qt{j}")
                    for j in range(4)
                ]
                kt_t = [
                    ktpool.tile([P, T], DT, tag=f"kt{j}", name=f"kt{j}")
                    for j in range(4)
                ]
                v_t = [
                    vpool.tile([P, HC * (DH + 1)], DT, tag=f"v{t}", name=f"v{t}")
                    for t in range(NT)
                ]
                yt_t = [
                    ytpool.tile([P, S], DT, tag=f"yt{j}", name=f"yt{j}")
                    for j in range(4)
                ]

                # ones columns of v_aug (col 64 of each 65-wide head block)
                for t in range(NT):
                    vv = v_t[t].rearrange("p (h d) -> p h d", h=HC)
                    nc.gpsimd.memset(vv[:, :, DH : DH + 1], 1.0)

                # output projection weights (DMA early; used late)
                wp_sb = wppool.tile([P, 4 * E], DT, tag="wp")
                nc.sync.dma_start(
                    wp_sb.rearrange("p (k o) -> p k o", k=4),
                    wpT.rearrange("(k p) o -> p k o", p=P),
                )

                # ---- v projection: v[t-tile, 512] = valueT.T @ WvT  (+bv) ----
                wv_tiles = []
                for k in range(KT):
                    w = wbuf.tile([P, EC], DT, tag="w")
                    nc.sync.dma_start(w[:, :], wvT[k * P : (k + 1) * P, :])
                    wv_tiles.append(w)
                vin = []
                for k in range(KT):
                    a = inbuf.tile([P, T], DT, tag="in")
                    nc.sync.dma_start(a[:, :], vT_in[k * P : (k + 1) * P, :])
                    vin.append(a)
                for t in range(NT):
                    ps = ps_sc.tile([P, TG * SCH], F32, tag="ps")
                    for k in range(KT):
                        nc.tensor.matmul(
                            ps[:, 0:EC],
                            lhsT=vin[k][:, t * P : (t + 1) * P],
                            rhs=wv_tiles[k][:, :],
                            start=(k == 0),
                            stop=(k == KT - 1),
                        )
                    vv = v_t[t].rearrange("p (h d) -> p h d", h=HC)
                    nc.vector.tensor_tensor(
                        vv[:, :, 0:DH],
                        ps[:, 0:EC].rearrange("p (h d) -> p h d", h=HC),
                        bvrep.rearrange("p (h d) -> p h d", h=HC),
                        ALU.add,
                    )

                # weight/input tiles for k/q projections (loaded once, used
                # interleaved with chunk-0 attention below)
                wk_tiles, kin, wq_tiles, qin = [], [], [], []
                for k in range(KT):
                    w = wbuf.tile([P, EC], DT, tag="w")
                    nc.sync.dma_start(w[:, :], wkT[k * P : (k + 1) * P, :])
                    wk_tiles.append(w)
                    a = inbuf.tile([P, T], DT, tag="in")
                    nc.sync.dma_start(a[:, :], kT_in[k * P : (k + 1) * P, :])
                    kin.append(a)
                for k in range(KT):
                    w = wbuf.tile([P, EC], DT, tag="w")
                    nc.sync.dma_start(w[:, :], wqT[k * P : (k + 1) * P, :])
                    wq_tiles.append(w)
                    a = inbuf.tile([P, S], DT, tag="in")
                    nc.sync.dma_start(a[:, :], qT_in[k * P : (k + 1) * P, :])
                    qin.append(a)

                def proj_unit(j, cc, w_tiles, act_tiles, bias_sb, dst):
                    def emit():
                        ps = ps_sc.tile([P, TG * SCH], F32, tag="ps")
                        for k in range(KT):
                            nc.tensor.matmul(
                                ps[:, 0:SCH],
                                lhsT=w_tiles[k][:, j * P : (j + 1) * P],
                                rhs=act_tiles[k][:, cc * SCH : (cc + 1) * SCH],
                                start=(k == 0),
                                stop=(k == KT - 1),
                            )
                        nc.vector.tensor_scalar(
                            dst[:, cc * SCH : (cc + 1) * SCH],
                            ps[:, 0:SCH],
                            bias_sb[:, j : j + 1],
                            None,
                            ALU.add,
                        )

                    return emit

                def proj_pair(j, w_tiles, act_tiles, bias_sb, dst):
                    for cc in range(NCH):
                        proj_unit(j, cc, w_tiles, act_tiles, bias_sb, dst)()

                def proj_units(j):
                    return [
                        proj_unit(j, cc, wk_tiles, kin, bk_sb, kt_t[j])
                        for cc in range(NCH)
                    ] + [
                        proj_unit(j, cc, wq_tiles, qin, bq_sb, qt_t[j])
                        for cc in range(NCH)
                    ]

                inv_scale = 1.0 / SCALE

                def attention(j, c, filler=None):
                    """Attention for head pair j, S-chunk c. Emission is
                    software-pipelined: scores(g) -> exp(g) -> y(g-1).
                    filler: list of closures emitted one-per-group (PE work
                    that fills gaps while ACT runs, e.g. deferred out-proj)."""
                    ps_ya = ps_y.tile([P, SCH], F32, tag="ps_ya")
                    ps_yb = ps_y.tile([P, SCH], F32, tag="ps_yb")
                    exps = {}

                    def emit_scores_exp(g):
                        ps_a = ps_sc.tile([P, TG * SCH], F32, tag="ps")
                        ps_b = ps_sc.tile([P, TG * SCH], F32, tag="ps")
                        for i in range(TG):
                            t = g * TG + i
                            nc.tensor.matmul(
                                ps_a[:, i * SCH : (i + 1) * SCH],
                                lhsT=kt_t[j][0:DH, t * P : (t + 1) * P],
                                rhs=qt_t[j][0:DH, c * SCH : (c + 1) * SCH],
                                start=True,
                                stop=True,
                            )
                            nc.tensor.matmul(
                                ps_b[:, i * SCH : (i + 1) * SCH],
                                lhsT=kt_t[j][DH:P, t * P : (t + 1) * P],
                                rhs=qt_t[j][DH:P, c * SCH : (c + 1) * SCH],
                                start=True,
                                stop=True,
                            )
                        exp_a = expool.tile([P, TG * SCH], DT, tag="exp")
                        exp_b = expool.tile([P, TG * SCH], DT, tag="exp")
                        nc.scalar.activation(
                            exp_a[:, :], ps_a[:, :], AF.Exp, scale=inv_scale
                        )
                        nc.scalar.activation(
                            exp_b[:, :], ps_b[:, :], AF.Exp, scale=inv_scale
                        )
                        exps[g] = (exp_a, exp_b)

                    def emit_y(g):
                        exp_a, exp_b = exps.pop(g)
                        for i in range(TG):
                            t = g * TG + i
                            va = v_t[t][
                                :, (2 * j) * (DH + 1) : (2 * j + 1) * (DH + 1)
                            ]
                            vb = v_t[t][
                                :, (2 * j + 1) * (DH + 1) : (2 * j + 2) * (DH + 1)
                            ]
                            nc.tensor.matmul(
                                ps_ya[0 : DH + 1, :],
                                lhsT=va,
                                rhs=exp_a[:, i * SCH : (i + 1) * SCH],
                                start=(t == 0),
                                stop=(t == NT - 1),
                            )
                            nc.tensor.matmul(
                                ps_yb[0 : DH + 1, :],
                                lhsT=vb,
                                rhs=exp_b[:, i * SCH : (i + 1) * SCH],
                                start=(t == 0),
                                stop=(t == NT - 1),
                            )

                    for g in range(NG):
                        emit_scores_exp(g)
                        if g >= 1:
                            emit_y(g - 1)
                        if filler:
                            filler.pop(0)()
                    emit_y(NG - 1)

                    # normalization: row 64 of ps_y* holds the softmax denom.
                    # Copy psum -> SBUF immediately (frees the y psum slots for
                    # the next pair), then recip/broadcast/mul off-path.
                    yun = norm.tile([P, 2 * SCH], F32, tag="yun")
                    nc.vector.tensor_copy(yun[0 : DH + 1, 0:SCH], ps_ya[0 : DH + 1, :])
                    nc.vector.tensor_copy(
                        yun[0 : DH + 1, SCH : 2 * SCH], ps_yb[0 : DH + 1, :]
                    )
                    nc.vector.reciprocal(
                        yun[DH : DH + 1, 0:SCH], yun[DH : DH + 1, 0:SCH]
                    )
                    nc.vector.reciprocal(
                        yun[DH : DH + 1, SCH : 2 * SCH],
                        yun[DH : DH + 1, SCH : 2 * SCH],
                    )
                    recrep = norm.tile([P, 2 * SCH], F32, tag="recrep")
                    nc.sync.dma_start(
                        recrep[0:DH, 0:SCH],
                        yun[DH : DH + 1, None, 0:SCH].broadcast_to([1, DH, SCH]),
                    )
                    nc.sync.dma_start(
                        recrep[0:DH, SCH : 2 * SCH],
                        yun[DH : DH + 1, None, SCH : 2 * SCH].broadcast_to(
                            [1, DH, SCH]
                        ),
                    )
                    nc.vector.tensor_tensor(
                        yt_t[j][0:DH, c * SCH : (c + 1) * SCH],
                        yun[0:DH, 0:SCH],
                        recrep[0:DH, 0:SCH],
                        ALU.mult,
                    )
                    stage_b = norm.tile([P, SCH], DT, tag="stageb")
                    nc.vector.tensor_tensor(
                        stage_b[0:DH, :],
                        yun[0:DH, SCH : 2 * SCH],
                        recrep[0:DH, SCH : 2 * SCH],
                        ALU.mult,
                    )
                    nc.sync.dma_start(
                        yt_t[j][DH:P, c * SCH : (c + 1) * SCH], stage_b[0:DH, :]
                    )

                def out_proj_unit(c, mi, n):
                    def emit():
                        ps = ps_sc.tile([P, TG * SCH], F32, tag="ps")
                        for kp in range(4):
                            nc.tensor.matmul(
                                ps[:, 0:EC],
                                lhsT=yt_t[kp][
                                    :, c * SCH + mi * P : c * SCH + (mi + 1) * P
                                ],
                                rhs=wp_sb[
                                    :, kp * E + n * EC : kp * E + (n + 1) * EC
                                ],
                                start=(kp == 0),
                                stop=(kp == 3),
                            )
                        ost = ostage.tile([P, EC], F32, tag="ost")
                        nc.vector.tensor_copy(ost[:, :], ps[:, 0:EC])
                        nc.sync.dma_start(
                            out_d[
                                c * SCH + mi * P : c * SCH + (mi + 1) * P,
                                n * EC : (n + 1) * EC,
                            ],
                            ost[:, :],
                        )

                    return emit

                for c in range(NCH):
                    for j in range(4):
                        if c == 0:
                            proj_pair(j, wk_tiles, kin, bk_sb, kt_t[j])
                            proj_pair(j, wq_tiles, qin, bq_sb, qt_t[j])
                        # deferred out-proj of the previous chunk fills PE
                        # gaps during the first pair's attention groups
                        filler = None
                        if c >= 1 and j == 0:
                            filler = [
                                out_proj_unit(c - 1, mi, n)
                                for mi in range(NCH)
                                for n in range(2)
                            ]
                        attention(j, c, filler=filler)
                    if c == NCH - 1:
                        for mi in range(NCH):
                            for n in range(2):
                                out_proj_unit(c, mi, n)()

                if debug_dumps:
                    dstage = dbgp.tile([P, S], F32, tag="dstage")
                    for j in range(4):
                        nc.vector.tensor_copy(dstage[:, 0:S], qt_t[j][:, :])
                        nc.sync.dma_start(dbg_qt[j], dstage[:, 0:S])
                    for j in range(4):
                        nc.vector.tensor_copy(dstage[:, 0:T], kt_t[j][:, :])
                        nc.sync.dma_start(dbg_kt[j], dstage[:, 0:T])
                    for t in range(NT):
                        nc.vector.tensor_copy(
                            dstage[:, 0 : HC * (DH + 1)], v_t[t][:, :]
                        )
                        nc.sync.dma_start(dbg_v[t], dstage[:, 0 : HC * (DH + 1)])
                    for j in range(4):
                        nc.vector.tensor_copy(dstage[:, 0:S], yt_t[j][:, :])
                        nc.sync.dma_start(dbg_yt[j], dstage[:, 0:S])

    nc.compile()
    return nc


def _get_nc():
    if "nc" not in _cache:
        _cache["nc"] = _build_nc()
    return _cache["nc"]


def _make_runner(nc):
    """Build a reusable sharded jit callable (mirrors bass2jax.run_bass_via_pjrt)."""
    import jax
    import numpy as np
    import concourse.mybir as mybir
    from concourse import bass2jax
    from concourse.bass2jax import _bass_exec_p, partition_id_tensor
    from jax.sharding import Mesh, PartitionSpec, NamedSharding
    from jax.experimental.shard_map import shard_map

    bass2jax.install_neuronx_cc_hook()

    partition_name = nc.partition_id_tensor.name if nc.partition_id_tensor else None
    in_names, out_names, out_avals, zero_shapes = [], [], [], []
    for alloc in nc.m.functions[0].allocations:
        if not isinstance(alloc, mybir.MemoryLocationSet):
            continue
        name = alloc.memorylocations[0].name
        if alloc.kind == "ExternalInput":
            if name != partition_name:
                in_names.append(name)
        elif alloc.kind == "ExternalOutput":
            out_names.append(name)
            shape = tuple(alloc.tensor_shape)
            dtype = mybir.dt.np(alloc.dtype)
            out_avals.append(jax.core.ShapedArray(shape, dtype))
            zero_shapes.append((shape, dtype))
    n_params = len(in_names)
    n_outs = len(out_avals)
    all_names = list(in_names) + list(out_names)
    if partition_name is not None:
        all_names.append(partition_name)

    def _body(*args):
        operands = list(args)
        if partition_name is not None:
            operands.append(partition_id_tensor())
        outs = _bass_exec_p.bind(
            *operands,
            out_avals=tuple(out_avals),
            in_names=tuple(all_names),
            out_names=tuple(out_names),
            lowering_input_output_aliases=(),
            sim_require_finite=True,
            sim_require_nnan=True,
            nc=nc,
        )
        return tuple(outs)

    devices = jax.devices()[:NC_]
    mesh = Mesh(np.asarray(devices), ("core",))
    in_specs = (PartitionSpec("core"),) * (n_params + n_outs)
    out_specs = (PartitionSpec("core"),) * n_outs
    sharded = jax.jit(
        shard_map(
            _body, mesh=mesh, in_specs=in_specs, out_specs=out_specs, check_rep=False
        ),
        keep_unused=True,
    )

    shd = NamedSharding(mesh, PartitionSpec("core"))

    def place(in_maps):
        """device_put the concatenated inputs + zero out-buffers once."""
        per_core = [[np.asarray(m[name]) for name in in_names] for m in in_maps]
        concat_in = [
            np.concatenate([per_core[c][i] for c in range(NC_)], axis=0)
            for i in range(n_params)
        ]
        concat_zeros = [
            np.zeros((NC_ * sh[0], *sh[1:]), dt) for sh, dt in zero_shapes
        ]
        args = [jax.device_put(a, shd) for a in concat_in + concat_zeros]
        jax.block_until_ready(args)
        return args

    def exec_dev(dev_args):
        out_arrs = sharded(*dev_args)
        jax.block_until_ready(out_arrs)
        return out_arrs

    def run(in_maps):
        out_arrs = exec_dev(place(in_maps))
        out_arrs = [np.asarray(a) for a in out_arrs]
        return [
            {
                name: out_arrs[i].reshape(NC_, *zero_shapes[i][0])[c]
                for i, name in enumerate(out_names)
            }
            for c in range(NC_)
        ]

    run.place = place
    run.exec_dev = exec_dev
    run.sharded = sharded
    run.in_names = in_names
    run.out_names = out_names
    run.zero_shapes = zero_shapes
    return run


def _shard_inputs(query, key, value, Wq, bq, Wk, bk, Wv, bv, Wp, bp):
    npdt = getattr(ml_dtypes, _DT_NAME) if _DT_NAME != "float32" else np.float32

    def cvt(x):
        return np.ascontiguousarray(x).astype(npdt)

    qT = [cvt(np.asarray(query[n]).T) for n in range(N)]
    kT = [cvt(np.asarray(key[n]).T) for n in range(N)]
    vT = [cvt(np.asarray(value[n]).T) for n in range(N)]
    in_maps = []
    for n in range(N):
        for hg in range(2):
            sl = slice(hg * EC, (hg + 1) * EC)
            in_maps.append(
                {
                    "qT_in": qT[n],
                    "kT_in": kT[n],
                    "vT_in": vT[n],
                    "wqT": cvt(np.asarray(Wq)[sl, :].T),
                    "wkT": cvt(np.asarray(Wk)[sl, :].T),
                    "wvT": cvt(np.asarray(Wv)[sl, :].T),
                    "wpT": cvt(np.asarray(Wp)[:, sl].T),
                    "bq": np.ascontiguousarray(np.asarray(bq)[sl], dtype=np.float32),
                    "bk": np.ascontiguousarray(np.asarray(bk)[sl], dtype=np.float32),
                    "bv": np.ascontiguousarray(np.asarray(bv)[sl], dtype=np.float32),
                }
            )
    return in_maps


def kernel(query, key, value, Wq, bq, Wk, bk, Wv, bv, Wp, bp):
    nc = _get_nc()
    if "runner" not in _cache:
        _cache["runner"] = _make_runner(nc)
    runner = _cache["runner"]
    in_maps = _shard_inputs(query, key, value, Wq, bq, Wk, bk, Wv, bv, Wp, bp)
    _cache["in_maps"] = in_maps
    results = runner(in_maps)
    bp32 = np.asarray(bp, dtype=np.float32)
    out = np.empty((N, S, E), dtype=np.float32)
    for n in range(N):
        out[n] = results[2 * n]["out"] + results[2 * n + 1]["out"] + bp32
    return out
